# revision 4
# baseline (speedup 1.0000x reference)
"""AvgPool2d-as-Toeplitz kernel for Trainium2 (8 NeuronCores, SPMD).

Reference computes out = (enc_x * mask) @ W.T where W is the dense
Toeplitz matrix of conv2d with kernel ones(C,C,KH,KW)/(KH*KW) over the
flattened zero-padded input (C=16, KH=KW=2, stride 2, pad 1, H=W=32).

Structure exploited:
  W[(co,oi,oj), (ci,i,j)] = 0.25  iff  i in {2oi, 2oi+1} and j in {2oj, 2oj+1}
— independent of co, summed over every ci. Hence with xm = enc_x * mask
viewed as [B, C, 34, 34]:

  out[b, co, oi, oj] = 0.25 * sum_{ci,ki,kj} xm[b, ci, 2oi+ki, 2oj+kj]

i.e. one channel-summed 2x2/stride-2 pooled [17,17] map per batch,
replicated across the 16 output channels. ~5 MB of I/O instead of the
342 MB dense weight.

Per-core plan (4 batches per core, batch-parallel across 8 cores):
  partitions = (b, ci) = 64, free = flattened 34x34 channel image.
  One packed input [64, 2376] per core: x image | mask image | E row,
  so a single DMA (one completion semaphore) feeds everything — walrus
  codegen allows at most one sync-wait per compute instruction.
  1. DMA packed input.
  2. DVE: v = x * mask.
  3. DVE: column-pair add  a[p, i, oj] = v[p, i, 2oj] + v[p, i, 2oj+1].
  4. DVE copy of the E block (keeps the PE's deps on the DVE semaphore).
     E[(b,ci), (b2,co)] = 0.25*(b==b2)  [64x64].
  5. PE: two accumulating matmuls:
       psum  = E.T @ a[:, 0::2, :]   (rows 2oi)
       psum += E.T @ a[:, 1::2, :]   (rows 2oi+1)
     One instruction pair performs the ci-sum, the row-pair sum, the
     0.25 scale and the broadcast over co.  psum = [ (b,co), (oi,oj) ].
  6. DVE copy PSUM -> SBUF, single DMA to out[4, 4624].
"""

import sys

import numpy as np

if "/opt/trn_rl_repo" not in sys.path:
    sys.path.insert(0, "/opt/trn_rl_repo")

B, C = 32, 16
HP = WP = 34
OH = OW = 17
IMG = HP * WP             # 1156
IN_DIM = C * IMG          # 18496
OUT_DIM = C * OH * OW     # 4624
N_CORES = 8
B_SH = B // N_CORES       # 4 batches per core
P = B_SH * C              # 64 partitions in use
PACK = 2 * IMG + P        # 2376

_PROGRAM = None


def _build_program():
    import concourse.bacc as bacc
    import concourse.mybir as mybir
    from concourse.tile import TileContext

    f32 = mybir.dt.float32
    nc = bacc.Bacc()

    xin = nc.declare_dram_parameter("xin", [P, PACK], f32, isOutput=False)
    out = nc.declare_dram_parameter("out", [B_SH, OUT_DIM], f32, isOutput=True)

    with TileContext(nc) as tc:
        with (
            tc.tile_pool(name="sb", bufs=1) as pool,
            tc.tile_pool(name="ps", bufs=1, space="PSUM") as psum_pool,
        ):
            it = pool.tile([P, PACK], f32)
            nc.sync.dma_start(it[:], xin[:, :])

            vt = pool.tile([P, IMG], f32)
            nc.vector.tensor_tensor(
                vt[:], it[:, 0:IMG], it[:, IMG:2 * IMG], mybir.AluOpType.mult
            )

            # column-pair add: [P, (i:34, oj:17)]
            at = pool.tile([P, HP * OW], f32)
            v4 = vt[:].rearrange("p (i oj t) -> p i oj t", i=HP, oj=OW, t=2)
            a_out = at[:].rearrange("p (i oj) -> p i oj", i=HP)
            nc.vector.tensor_tensor(
                a_out, v4[:, :, :, 0], v4[:, :, :, 1], mybir.AluOpType.add
            )

            # E block through DVE so the matmul's deps collapse to one sem
            et = pool.tile([P, P], f32)
            nc.vector.tensor_copy(et[:], it[:, 2 * IMG:PACK])

            # row-pair + channel-sum + 0.25 + co-broadcast via PE
            pt = psum_pool.tile([P, OH * OW], f32)
            a3 = at[:].rearrange("p (i oj) -> p i oj", i=HP)
            nc.tensor.matmul(pt[:], et[:], a3[:, 0:HP:2, :], start=True, stop=False)
            nc.tensor.matmul(pt[:], et[:], a3[:, 1:HP:2, :], start=False, stop=True)

            ot = pool.tile([P, OH * OW], f32)
            nc.vector.tensor_copy(ot[:], pt[:])
            nc.sync.dma_start(
                out[:, :].rearrange("b (co s) -> (b co) s", co=C), ot[:]
            )
    nc.compile()
    return nc


def _get_program():
    global _PROGRAM
    if _PROGRAM is None:
        _PROGRAM = _build_program()
    return _PROGRAM


def _block_e() -> np.ndarray:
    # E[(b,ci), (b2,co)] = 0.25 iff b == b2
    return (np.kron(np.eye(B_SH, dtype=np.float32), np.ones((C, C), np.float32))
            * np.float32(0.25))


def _pack_inputs(enc_x: np.ndarray, mask: np.ndarray) -> list:
    e = _block_e()
    in_maps = []
    for i in range(N_CORES):
        sl = slice(i * B_SH, (i + 1) * B_SH)
        xr = enc_x[sl].reshape(P, IMG)
        mr = mask[sl].reshape(P, IMG)
        packed = np.concatenate([xr, mr, e], axis=1)
        in_maps.append({"xin": np.ascontiguousarray(packed, dtype=np.float32)})
    return in_maps


def _run(enc_x: np.ndarray, mask: np.ndarray, **spmd_kwargs):
    from concourse.bass_utils import run_bass_kernel_spmd

    nc = _get_program()
    in_maps = _pack_inputs(enc_x, mask)
    res = run_bass_kernel_spmd(nc, in_maps, list(range(N_CORES)), **spmd_kwargs)
    out = np.concatenate([res.results[i]["out"] for i in range(N_CORES)], axis=0)
    return out, res


def kernel(enc_x, weight=None, mask=None, **_unused):
    enc_x = np.asarray(enc_x, dtype=np.float32)
    mask = np.asarray(mask, dtype=np.float32)
    assert enc_x.shape == (B, IN_DIM), enc_x.shape
    out, _ = _run(enc_x, mask)
    return out


# revision 6
# speedup vs baseline: 1.1616x; 1.1616x over previous
"""AvgPool2d-as-Toeplitz kernel for Trainium2 (8 NeuronCores, SPMD).

Reference computes out = (enc_x * mask) @ W.T where W is the dense
Toeplitz matrix of conv2d with kernel ones(C,C,KH,KW)/(KH*KW) over the
flattened zero-padded input (C=16, KH=KW=2, stride 2, pad 1, H=W=32),
and mask zeroes the 1-pixel padding ring of each 34x34 channel image.

Structure exploited:
  W[(co,oi,oj), (ci,i,j)] = 0.25  iff  i in {2oi, 2oi+1} and j in {2oj, 2oj+1}
— independent of co, summed over every ci. Hence with x viewed as
[B, C, 34, 34] and the mask ring folded in structurally (pooling windows
simply never read the masked border rows/columns):

  out[b, co, oi, oj] = 0.25 * sum_ci sum_window x[b, ci, i, j]
       over i in {2oi, 2oi+1} ∩ [1,32],  j in {2oj, 2oj+1} ∩ [1,32]

i.e. one channel-summed 2x2/stride-2 pooled [17,17] map per batch,
replicated across the 16 output channels. ~2.4 MB of input instead of
the 342 MB dense weight + 2.4 MB mask.

Per-core plan (4 batches per core, batch-parallel across 8 cores):
  partitions = (b, ci) = 64, free = flattened 34x34 channel image.
  1. DMA x-shard [64, 1156]; DMA E [64, 64] in parallel.
  2. Column-pair stage -> a[p, i, oj] (i: 34, oj: 17):
       DVE : a[:, :, 1:16] = x[:, :, 2:32:2] + x[:, :, 3:33:2]
       GPS : a[:, :, 0]    = x[:, :, 1]        (col 0 masked)
       GPS : a[:, :, 16]   = x[:, :, 32]       (col 33 masked)
  3. Row-pair stage -> a2[p, oi, oj]:
       DVE : a2[:, 1:16, :] = a[:, 2:32:2, :] + a[:, 3:33:2, :]
       GPS : a2[:, 0, :]    = a[:, 1, :]       (row 0 masked)
       GPS : a2[:, 16, :]   = a[:, 32, :]      (row 33 masked)
  4. PE: single accumulating matmul with constant block matrix
     E[(b,ci), (b2,co)] = 0.25*(b==b2):
       psum[(b,co), (oi,oj)] = E.T @ a2
     performing the ci-sum, the 0.25 scale and the broadcast over co.
  5. DVE copy PSUM -> SBUF, single DMA to out[4, 4624].
"""

import sys

import numpy as np

if "/opt/trn_rl_repo" not in sys.path:
    sys.path.insert(0, "/opt/trn_rl_repo")

B, C = 32, 16
HP = WP = 34
OH = OW = 17
IMG = HP * WP             # 1156
IN_DIM = C * IMG          # 18496
OUT_DIM = C * OH * OW     # 4624
N_CORES = 8
B_SH = B // N_CORES       # 4 batches per core
P = B_SH * C              # 64 partitions in use

_PROGRAM = None


def _build_program():
    import concourse.bacc as bacc
    import concourse.mybir as mybir
    from concourse.tile import TileContext

    f32 = mybir.dt.float32
    nc = bacc.Bacc()

    x = nc.declare_dram_parameter("x", [B_SH, IN_DIM], f32, isOutput=False)
    e = nc.declare_dram_parameter("e", [P, P], f32, isOutput=False)
    out = nc.declare_dram_parameter("out", [B_SH, OUT_DIM], f32, isOutput=True)

    with TileContext(nc) as tc:
        with (
            tc.tile_pool(name="sb", bufs=1) as pool,
            tc.tile_pool(name="ps", bufs=1, space="PSUM") as psum_pool,
        ):
            xt = pool.tile([P, IMG], f32)
            et = pool.tile([P, P], f32)
            nc.sync.dma_start(xt[:], x[:, :].rearrange("b (c f) -> (b c) f", c=C))
            nc.sync.dma_start(et[:], e[:, :])

            x3 = xt[:].rearrange("p (i j) -> p i j", i=HP)

            # column-pair stage: a[p, i, oj]
            at = pool.tile([P, HP * OW], f32)
            a3 = at[:].rearrange("p (i oj) -> p i oj", i=HP)
            nc.vector.tensor_tensor(
                a3[:, :, 1:16], x3[:, :, 2:32:2], x3[:, :, 3:33:2],
                mybir.AluOpType.add,
            )
            nc.gpsimd.tensor_copy(a3[:, :, 0], x3[:, :, 1])
            nc.gpsimd.tensor_copy(a3[:, :, 16], x3[:, :, 32])

            # row-pair stage: a2[p, oi, oj]
            a2t = pool.tile([P, OH * OW], f32)
            a23 = a2t[:].rearrange("p (oi oj) -> p oi oj", oi=OH)
            nc.vector.tensor_tensor(
                a23[:, 1:16, :], a3[:, 2:32:2, :], a3[:, 3:33:2, :],
                mybir.AluOpType.add,
            )
            nc.gpsimd.tensor_copy(a23[:, 0, :], a3[:, 1, :])
            nc.gpsimd.tensor_copy(a23[:, 16, :], a3[:, 32, :])

            # ci-sum + 0.25 + co-broadcast via PE
            pt = psum_pool.tile([P, OH * OW], f32)
            nc.tensor.matmul(pt[:], et[:], a2t[:], start=True, stop=True)

            ot = pool.tile([P, OH * OW], f32)
            nc.vector.tensor_copy(ot[:], pt[:])
            nc.sync.dma_start(
                out[:, :].rearrange("b (co s) -> (b co) s", co=C), ot[:]
            )
    nc.compile()
    return nc


def _get_program():
    global _PROGRAM
    if _PROGRAM is None:
        _PROGRAM = _build_program()
    return _PROGRAM


def _block_e() -> np.ndarray:
    # E[(b,ci), (b2,co)] = 0.25 iff b == b2
    return (np.kron(np.eye(B_SH, dtype=np.float32), np.ones((C, C), np.float32))
            * np.float32(0.25))


def _run(enc_x: np.ndarray, mask: np.ndarray = None, **spmd_kwargs):
    from concourse.bass_utils import run_bass_kernel_spmd

    nc = _get_program()
    e = _block_e()
    in_maps = []
    for i in range(N_CORES):
        sl = slice(i * B_SH, (i + 1) * B_SH)
        in_maps.append(
            {"x": np.ascontiguousarray(enc_x[sl], dtype=np.float32), "e": e}
        )
    res = run_bass_kernel_spmd(nc, in_maps, list(range(N_CORES)), **spmd_kwargs)
    out = np.concatenate([res.results[i]["out"] for i in range(N_CORES)], axis=0)
    return out, res


def kernel(enc_x, weight=None, mask=None, **_unused):
    enc_x = np.asarray(enc_x, dtype=np.float32)
    assert enc_x.shape == (B, IN_DIM), enc_x.shape
    out, _ = _run(enc_x)
    return out


# revision 12
# speedup vs baseline: 1.1943x; 1.0281x over previous
"""AvgPool2d-as-Toeplitz kernel for Trainium2 (8 NeuronCores, SPMD).

Reference computes out = (enc_x * mask) @ W.T where W is the dense
Toeplitz matrix of conv2d with kernel ones(C,C,KH,KW)/(KH*KW) over the
flattened zero-padded input (C=16, KH=KW=2, stride 2, pad 1, H=W=32),
and mask zeroes the 1-pixel padding ring of each 34x34 channel image.

Structure exploited:
  W[(co,oi,oj), (ci,i,j)] = 0.25  iff  i in {2oi, 2oi+1} and j in {2oj, 2oj+1}
— independent of co, summed over every ci. Hence with x viewed as
[B, C, 34, 34] and the mask ring folded in structurally (pooling windows
simply never read the masked border rows/columns):

  out[b, co, oi, oj] = 0.25 * sum_ci sum_window x[b, ci, i, j]
       over i in {2oi, 2oi+1} ∩ [1,32],  j in {2oj, 2oj+1} ∩ [1,32]

i.e. one channel-summed 2x2/stride-2 pooled [17,17] map per batch,
replicated across the 16 output channels. ~2.4 MB of input instead of
the 342 MB dense weight + 2.4 MB mask.

Per-core plan (4 batches per core, batch-parallel across 8 cores):
  partitions = (b, ci) = 64, free = flattened 34x34 channel image.
  0. GPSIMD builds E[(b,ci), (b2,co)] = 0.25*(b==b2) with 5 memsets
     (runs under the input DMA).
  1. x-shard [64, 1156] DMA'd in two halves (image rows 0-16 / 17-33)
     on the two HWDGE rings (sync + scalar engines).
  2. Column-pair stage -> a[p, i, oj] (i: 34, oj: 17), DVE:
       a[:, :, 1:16]   = x[:, :, 2:32:2] + x[:, :, 3:33:2]   (two halves)
       a[:, :, {0,16}] = x[:, :, {1,32}]     (masked border cols, 1 copy)
  3. Row-pair stage -> a2[p, oi, oj], DVE:
       a2[:, 1:16, :]   = a[:, 2:32:2, :] + a[:, 3:33:2, :]
       a2[:, {0,16}, :] = a[:, {1,32}, :]    (masked border rows, 1 copy)
  4. PE: single matmul  psum[(b,co), (oi,oj)] = E.T @ a2
     performing the ci-sum, the 0.25 scale and the broadcast over co.
  5. DVE copy PSUM -> SBUF, single DMA to out[4, 4624].
"""

import sys

import numpy as np

if "/opt/trn_rl_repo" not in sys.path:
    sys.path.insert(0, "/opt/trn_rl_repo")

B, C = 32, 16
HP = WP = 34
OH = OW = 17
IMG = HP * WP             # 1156
IN_DIM = C * IMG          # 18496
OUT_DIM = C * OH * OW     # 4624
N_CORES = 8
B_SH = B // N_CORES       # 4 batches per core
P = B_SH * C              # 64 partitions in use
ROWS0 = 17                # rows in first DMA half

_PROGRAM = None


def _build_program():
    import concourse.bacc as bacc
    import concourse.mybir as mybir
    from concourse.tile import TileContext

    f32 = mybir.dt.float32
    nc = bacc.Bacc()

    x = nc.declare_dram_parameter("x", [B_SH, IN_DIM], f32, isOutput=False)
    out = nc.declare_dram_parameter("out", [B_SH, OUT_DIM], f32, isOutput=True)
    xv = x[:, :].rearrange("b (c f) -> (b c) f", c=C)   # [64, 1156]

    with TileContext(nc) as tc:
        with (
            tc.tile_pool(name="sb", bufs=1) as pool,
            tc.tile_pool(name="ps", bufs=1, space="PSUM") as psum_pool,
        ):
            # E built on-device while the DMA is in flight:
            # E[p, (qb,qc)] = 0.25 iff p//16 == qb, i.e. 0 <= p - 16*qb <= 15
            et = pool.tile([P, P], f32)
            nc.gpsimd.memset(et[:], 0.25)
            e3 = et[:].rearrange("p (qb qc) -> p qb qc", qb=B_SH)
            nc.gpsimd.affine_select(
                e3, e3, [[-C, B_SH], [0, C]], mybir.AluOpType.is_ge, 0.0,
                base=0, channel_multiplier=1,
            )
            nc.gpsimd.affine_select(
                e3, e3, [[C, B_SH], [0, C]], mybir.AluOpType.is_ge, 0.0,
                base=C - 1, channel_multiplier=-1,
            )

            xt = pool.tile([P, IMG], f32)
            FH = ROWS0 * WP  # 578
            nc.sync.dma_start(xt[:, 0:FH], xv[:, 0:FH])
            nc.scalar.dma_start(xt[:, FH:IMG], xv[:, FH:IMG])

            x3 = xt[:].rearrange("p (i j) -> p i j", i=HP)

            # column-pair stage: a[p, i, oj]
            at = pool.tile([P, HP * OW], f32)
            a3 = at[:].rearrange("p (i oj) -> p i oj", i=HP)
            nc.vector.tensor_tensor(
                a3[:, 0:ROWS0, 1:16],
                x3[:, 0:ROWS0, 2:32:2], x3[:, 0:ROWS0, 3:33:2],
                mybir.AluOpType.add,
            )
            nc.vector.tensor_tensor(
                a3[:, ROWS0:HP, 1:16],
                x3[:, ROWS0:HP, 2:32:2], x3[:, ROWS0:HP, 3:33:2],
                mybir.AluOpType.add,
            )
            # border cols 0 and 16 <- x cols 1 and 32, one strided copy
            nc.vector.tensor_copy(a3[:, :, 0:17:16], x3[:, :, 1:33:31])

            # row-pair stage: a2[p, oi, oj]
            a2t = pool.tile([P, OH * OW], f32)
            a23 = a2t[:].rearrange("p (oi oj) -> p oi oj", oi=OH)
            nc.vector.tensor_tensor(
                a23[:, 1:16, :], a3[:, 2:32:2, :], a3[:, 3:33:2, :],
                mybir.AluOpType.add,
            )
            # border rows 0 and 16 <- a rows 1 and 32, one strided copy
            nc.vector.tensor_copy(a23[:, 0:17:16, :], a3[:, 1:33:31, :])

            # ci-sum + 0.25 + co-broadcast via PE
            pt = psum_pool.tile([P, OH * OW], f32)
            nc.tensor.matmul(pt[:], et[:], a2t[:], start=True, stop=True)

            ot = pool.tile([P, OH * OW], f32)
            nc.vector.tensor_copy(ot[:], pt[:])
            nc.sync.dma_start(
                out[:, :].rearrange("b (co s) -> (b co) s", co=C), ot[:]
            )
    nc.compile()
    return nc


def _get_program():
    global _PROGRAM
    if _PROGRAM is None:
        _PROGRAM = _build_program()
    return _PROGRAM


def _run(enc_x: np.ndarray, mask: np.ndarray = None, **spmd_kwargs):
    from concourse.bass_utils import run_bass_kernel_spmd

    nc = _get_program()
    in_maps = []
    for i in range(N_CORES):
        sl = slice(i * B_SH, (i + 1) * B_SH)
        in_maps.append({"x": np.ascontiguousarray(enc_x[sl], dtype=np.float32)})
    res = run_bass_kernel_spmd(nc, in_maps, list(range(N_CORES)), **spmd_kwargs)
    out = np.concatenate([res.results[i]["out"] for i in range(N_CORES)], axis=0)
    return out, res


def kernel(enc_x, weight=None, mask=None, **_unused):
    enc_x = np.asarray(enc_x, dtype=np.float32)
    assert enc_x.shape == (B, IN_DIM), enc_x.shape
    out, _ = _run(enc_x)
    return out
